# revision 6
# baseline (speedup 1.0000x reference)
"""Trainium2 Bass kernel for retrieval-KNN soft attention (nn_NONA_54915451847255).

out = clip(softmax(-||x_i - x_n_j||_2, diag-masked) @ y_n, 0, 1)

Sharding: queries row-sharded across 8 cores; x_n / y_n replicated but ROLLED by
-core*1024 rows on the host so the self-match diagonal always falls in local key
tiles 0..7 -> the SPMD instruction stream is core-independent.

Math per core (1024 queries, 8192 keys), computed transposed (S_T[j,i]):
  psum = sum_d xnT[d,j] * (-2 x[d,i])  + 1*(qnorm_i - 512)      (PE, float32r)
  z    = psum + (knorm_j + 512)                                 (ACT bias / DVE on diag tiles)
  P_T  = exp(-exp(0.5 * ln(z)))  = exp(-sqrt(z))                (ACT, one table set)
  out_T[c,i] = sum_j y_aug[j,c] * P_T[j,i],  y_aug = [y_n | 1]  (PE, col C = rowsum)
  out[i,c] = clip(out_T[c,i] / out_T[C,i], 0, 1)
"""
import numpy as np

import concourse.bacc as bacc
import concourse.tile as tile
from concourse import mybir
from concourse.bass_utils import run_bass_kernel_spmd

F32 = mybir.dt.float32
F32R = mybir.dt.float32r
AF = mybir.ActivationFunctionType
ALU = mybir.AluOpType

N, D, C = 8192, 512, 100
NCORES = 8
QPC = N // NCORES          # 1024 queries per core
NKT = N // 128             # 64 key tiles
NQG = QPC // 512           # 2 query groups of 512
NDC = D // 128             # 4 contraction chunks
CA = C + 1                 # y augmented with ones column


def build_nc():
    nc = bacc.Bacc("TRN2", target_bir_lowering=False, debug=False)
    xq_d = nc.dram_tensor("xq", [QPC, D], F32, kind="ExternalInput").ap()
    xk_d = nc.dram_tensor("xk", [N, D], F32, kind="ExternalInput").ap()
    yk_d = nc.dram_tensor("yk", [N, C], F32, kind="ExternalInput").ap()
    id_d = nc.dram_tensor("ident", [128, 128], F32, kind="ExternalInput").ap()
    mk_d = nc.dram_tensor("dmask", [128, 128], F32, kind="ExternalInput").ap()
    out_d = nc.dram_tensor("out", [QPC, C], F32, kind="ExternalOutput").ap()

    with tile.TileContext(nc) as tc:
        with (
            tc.tile_pool(name="const", bufs=1) as constp,
            tc.tile_pool(name="ybank", bufs=1) as ybankp,
            tc.tile_pool(name="yraw", bufs=4) as yrawp,
            tc.tile_pool(name="xqraw", bufs=3) as xqrawp,
            tc.tile_pool(name="xt", bufs=1) as xtp,
            tc.tile_pool(name="xk", bufs=4) as xkp,
            tc.tile_pool(name="xnt", bufs=4) as xntp,
            tc.tile_pool(name="sq", bufs=3) as sqp,
            tc.tile_pool(name="kn", bufs=6) as knp,
            tc.tile_pool(name="act", bufs=4) as actp,
            tc.tile_pool(name="pt", bufs=4) as ptp,
            tc.tile_pool(name="fin", bufs=4) as finp,
            tc.tile_pool(name="trps", bufs=4, space="PSUM") as trps,
            tc.tile_pool(name="stps", bufs=2, space="PSUM") as stps,
            tc.tile_pool(name="outps", bufs=1, space="PSUM") as outps,
        ):
            ident = constp.tile([128, 128], F32)
            nc.sync.dma_start(ident[:], id_d)
            dmask = constp.tile([128, 128], F32R)
            dmask_f = constp.tile([128, 128], F32)
            nc.sync.dma_start(dmask_f[:], mk_d)
            nc.vector.tensor_copy(dmask[:], dmask_f[:])

            ones_aug = constp.tile([1, 128], F32R)
            ones_f = constp.tile([1, 128], F32)
            nc.vector.memset(ones_f[:], 1.0)
            nc.vector.tensor_copy(ones_aug[:], ones_f[:])

            # ---- y bank: [128, 64*101] f32r, col 100 of each chunk = 1.0 ----
            ybank = ybankp.tile([128, NKT * CA], F32R)
            for t in range(NKT):
                yr = yrawp.tile([128, C], F32)
                nc.sync.dma_start(yr[:], yk_d[t * 128:(t + 1) * 128, :])
                nc.vector.tensor_copy(ybank[:, t * CA:t * CA + C], yr[:])
            ones_col = ybank[:].rearrange("p (t c) -> p t c", c=CA)[:, :, C:CA]
            ones64 = constp.tile([128, NKT], F32)
            nc.vector.memset(ones64[:], 1.0)
            nc.vector.tensor_copy(ones_col, ones64[:].rearrange("p (t o) -> p t o", o=1))

            # ---- xT: [128, 4 * 1024] f32r = -2 * x^T, and qn_row = qnorm-512 ----
            xT = xtp.tile([128, NDC * QPC], F32R)
            qn_row = constp.tile([1, QPC], F32R)
            for m in range(QPC // 128):
                xqt = xqrawp.tile([128, D], F32)
                nc.sync.dma_start(xqt[:], xq_d[m * 128:(m + 1) * 128, :])
                sqt = sqp.tile([128, D], F32)
                nc.vector.tensor_mul(sqt[:], xqt[:], xqt[:])
                qn = knp.tile([128, 1], F32)
                nc.vector.reduce_sum(qn[:], sqt[:], axis=mybir.AxisListType.X)
                qnc = knp.tile([128, 1], F32)
                nc.vector.tensor_scalar_add(qnc[:], qn[:], -512.0)
                # transpose qnorm column -> row slice
                ptr = trps.tile([1, 128], F32, tag="tr")
                nc.tensor.transpose(ptr[:], qnc[:, 0:1], ident[:])
                nc.vector.tensor_copy(qn_row[0:1, m * 128:(m + 1) * 128], ptr[:])
                for kd in range(NDC):
                    ptx = trps.tile([128, 128], F32, tag="tr")
                    nc.tensor.transpose(ptx[:], xqt[:, kd * 128:(kd + 1) * 128], ident[:])
                    nc.vector.tensor_scalar_mul(
                        xT[:, kd * QPC + m * 128: kd * QPC + (m + 1) * 128], ptx[:], -2.0)

            # ---- persistent output accumulators [101, 512] per query group ----
            outp = [outps.tile([CA, 512], F32, name=f"outp{qg}") for qg in range(NQG)]

            # ---- main loop over key tiles ----
            for kt in range(NKT):
                xkt = xkp.tile([128, D], F32)
                nc.sync.dma_start(xkt[:], xk_d[kt * 128:(kt + 1) * 128, :])
                sqt = sqp.tile([128, D], F32)
                nc.vector.tensor_mul(sqt[:], xkt[:], xkt[:])
                kn = knp.tile([128, 1], F32)
                nc.vector.reduce_sum(kn[:], sqt[:], axis=mybir.AxisListType.X)
                kb = knp.tile([128, 1], F32)
                nc.vector.tensor_scalar_add(kb[:], kn[:], 512.0)

                xnT = xntp.tile([128, D], F32R)
                for kd in range(NDC):
                    ptx = trps.tile([128, 128], F32, tag="tr")
                    nc.tensor.transpose(ptx[:], xkt[:, kd * 128:(kd + 1) * 128], ident[:])
                    nc.vector.tensor_copy(xnT[:, kd * 128:(kd + 1) * 128], ptx[:])

                for qg in range(NQG):
                    st = stps.tile([128, 512], F32)
                    for kd in range(NDC):
                        nc.tensor.matmul(
                            st[:], xnT[:, kd * 128:(kd + 1) * 128],
                            xT[:, kd * QPC + qg * 512: kd * QPC + qg * 512 + 512],
                            start=(kd == 0), stop=False)
                    nc.tensor.matmul(
                        st[:], ones_aug[0:1, :], qn_row[0:1, qg * 512:qg * 512 + 512],
                        start=False, stop=True)

                    diag = kt < 8 and qg == kt // 4
                    s1 = actp.tile([128, 512], F32)
                    if diag:
                        # z = psum + (knorm+512), clamped away from 0 under the diagonal
                        nc.vector.tensor_scalar(st[:], st[:], kb[:, 0:1], 350.0,
                                                ALU.add, ALU.max)
                        nc.scalar.activation(s1[:], st[:], AF.Ln)
                    else:
                        nc.scalar.activation(s1[:], st[:], AF.Ln, bias=kb[:, 0:1])
                    s2 = actp.tile([128, 512], F32)
                    nc.scalar.activation(s2[:], s1[:], AF.Exp, scale=0.5)
                    pt = ptp.tile([128, 512], F32R)
                    nc.scalar.activation(pt[:], s2[:], AF.Exp, scale=-1.0)
                    if diag:
                        off = (kt % 4) * 128
                        nc.vector.tensor_mul(pt[:, off:off + 128],
                                             pt[:, off:off + 128], dmask[:])
                    nc.tensor.matmul(outp[qg][:], ybank[:, kt * CA:(kt + 1) * CA],
                                     pt[:], start=(kt == 0), stop=(kt == NKT - 1))

            # ---- finalize: transpose back, normalize, clip, store ----
            for qg in range(NQG):
                osb = finp.tile([CA, 512], F32)
                nc.vector.tensor_copy(osb[:], outp[qg][:])
                for t in range(4):
                    ptf = trps.tile([128, CA], F32, tag="tr")
                    nc.tensor.transpose(ptf[:], osb[:, t * 128:(t + 1) * 128],
                                        ident[0:CA, 0:CA])
                    rc = knp.tile([128, 1], F32)
                    nc.vector.reciprocal(rc[:], ptf[:, C:CA])
                    ob = finp.tile([128, C], F32)
                    nc.vector.tensor_scalar(ob[:], ptf[:, 0:C], rc[:, 0:1], 1.0,
                                            ALU.mult, ALU.min)
                    nc.sync.dma_start(
                        out_d[qg * 512 + t * 128: qg * 512 + (t + 1) * 128, :], ob[:])

    nc.compile()
    return nc


_NC_CACHE = []


def kernel(x, x_n, y_n):
    x = np.ascontiguousarray(np.asarray(x, dtype=np.float32))
    x_n = np.ascontiguousarray(np.asarray(x_n, dtype=np.float32))
    y_n = np.ascontiguousarray(np.asarray(y_n, dtype=np.float32))
    if not _NC_CACHE:
        _NC_CACHE.append(build_nc())
    nc = _NC_CACHE[0]

    ident = np.eye(128, dtype=np.float32)
    dmask = (1.0 - np.eye(128, dtype=np.float32))
    in_maps = []
    for c in range(NCORES):
        s = c * QPC
        in_maps.append({
            "xq": x[s:s + QPC],
            "xk": np.roll(x_n, -s, axis=0),
            "yk": np.roll(y_n, -s, axis=0),
            "ident": ident,
            "dmask": dmask,
        })
    import os
    trace = bool(int(os.environ.get("KERNEL_TRACE", "0")))
    res = run_bass_kernel_spmd(nc, in_maps, core_ids=list(range(NCORES)),
                               trace=trace)
    if trace:
        print("exec_time_ns:", res.exec_time_ns,
              "mean:", res.mean_exec_time_ns, flush=True)
        if res.instructions_and_trace:
            print("trace:", res.instructions_and_trace[1], flush=True)
    out = np.concatenate([r["out"] for r in res.results], axis=0)
    return out.astype(np.float32)
